# revision 1
# baseline (speedup 1.0000x reference)
"""Trainium2 Bass kernel: batched causal attention (B=4, S=4096, E=256, f32).

Sharding: 2 cores per batch element (4 pairs). Within a pair, the key/value
rows are split even/odd at 128-row tile granularity; both cores process all
4096 query rows of their batch against their 2048 K/V rows.  This keeps the
SPMD instruction stream identical across cores (only data differs) and
perfectly load-balances the causal structure.  Partial (P@V, rowsum) results
are merged across each pair with per-pair ReduceScatters (bf16 payload),
after which each core normalizes and writes half the batch rows.

Compute layout (per core):
  X^T, Z^T via PE transposes (bf16) -> Q^T = WqT @ X^T (scaled 1/sqrt(E),
  +bq), K^T = WkT @ Z^T (bk dropped: softmax shift-invariant), V = Z^T
  (stationary) @ WvT (bv folded in at the end: attn rows sum to 1).
  Scores per tile are computed transposed: S^T[k,q] = K^T(stationary) . Q^T,
  so exp(PSUM)->SBUF directly yields P^T (bf16) for the PV matmul.
  Rowsums via a ones-stationary matmul accumulated in PSUM.
  Pairs are processed most-expensive-first so collectives overlap compute.
"""

import numpy as np

B = 4
S = 4096
E = 256
SK = S // 2          # K/V rows per core
KT = SK // 128       # 16 local k-tiles
NCHUNK = S // 512    # 8 q-chunks of 512
F = 512              # q free dim per chunk
NPOST = NCHUNK // 2  # post-phase chunks per core

_COMPILED = {}


def _build():
    import concourse.bass as bass
    import concourse.tile as tile
    from concourse import mybir, bacc
    from concourse.masks import make_identity

    f32 = mybir.dt.float32
    bf16 = mybir.dt.bfloat16
    Exp = mybir.ActivationFunctionType.Exp
    Copy = mybir.ActivationFunctionType.Copy
    Ident = mybir.ActivationFunctionType.Identity

    nc = bacc.Bacc("TRN2", target_bir_lowering=False, debug=False,
                   enable_asserts=True, num_devices=8)

    x_ext = nc.dram_tensor("x", [S, E], f32, kind="ExternalInput")
    z_ext = nc.dram_tensor("z", [SK, E], f32, kind="ExternalInput")
    wq_ext = nc.dram_tensor("wq", [E, E], f32, kind="ExternalInput")
    wk_ext = nc.dram_tensor("wk", [E, E], f32, kind="ExternalInput")
    wv_ext = nc.dram_tensor("wv", [E, E], f32, kind="ExternalInput")
    bqs_ext = nc.dram_tensor("bqs", [E], f32, kind="ExternalInput")  # bq/sqrt(E)
    bv_ext = nc.dram_tensor("bv", [E], f32, kind="ExternalInput")
    masks_ext = nc.dram_tensor("masks", [2, 128, F], f32, kind="ExternalInput")
    ones_ext = nc.dram_tensor("ones", [128, 128], f32, kind="ExternalInput")
    out_ext = nc.dram_tensor("out", [S // 2, E], f32, kind="ExternalOutput")

    with tile.TileContext(nc) as tc:
        with tc.tile_pool(name="singles", bufs=1) as singles, \
             tc.tile_pool(name="dram", bufs=1, space="DRAM") as dram:
            # ---- constants -------------------------------------------------
            ident_bf = singles.tile([128, 128], bf16)
            make_identity(nc, ident_bf[:])
            ones_r = singles.tile([128, 128], bf16)
            nc.gpsimd.dma_start(out=ones_r[:], in_=ones_ext[:])
            maskt = singles.tile([128, 2, F], bf16)
            nc.gpsimd.dma_start(out=maskt[:], in_=masks_ext.ap().rearrange("m p f -> p m f"))
            bqs = singles.tile([128, 2], f32)
            for ft in range(2):
                nc.sync.dma_start(out=bqs[:, ft:ft + 1],
                                  in_=bqs_ext[128 * ft:128 * (ft + 1)].rearrange("(p one) -> p one", one=1))
            bv_bc = singles.tile([128, E], f32)
            nc.sync.dma_start(
                out=bv_bc[:],
                in_=bass.AP(tensor=bv_ext, offset=0, ap=[[0, 128], [1, E]]))

            # ---- weights: W^T[e', f] in SBUF (bf16), via PE transposes -----
            wT = {}
            with tc.tile_pool(name="wload", bufs=2) as wload, \
                 tc.tile_pool(name="ps_w", bufs=2, space="PSUM") as ps_w:
                for wname, wext in (("q", wq_ext), ("k", wk_ext), ("v", wv_ext)):
                    for et in range(2):
                        wT[wname, et] = singles.tile([128, E], bf16, name=f"wT_{wname}{et}")
                    for ft in range(2):
                        wnat = wload.tile([128, E], bf16, name="wnat")
                        nc.gpsimd.dma_start(out=wnat[:],
                                            in_=wext[128 * ft:128 * (ft + 1), :])
                        pst = ps_w.tile([128, E], bf16, name="pstw")
                        for et in range(2):
                            nc.tensor.transpose(pst[:, 128 * et:128 * (et + 1)],
                                                wnat[:, 128 * et:128 * (et + 1)],
                                                ident_bf[:])
                        for et in range(2):
                            nc.vector.tensor_copy(
                                out=wT[wname, et][:, 128 * ft:128 * (ft + 1)],
                                in_=pst[:, 128 * et:128 * (et + 1)])

            # ---- big persistent SBUF tensors -------------------------------
            qT = [singles.tile([128, S], bf16, name=f"qT{i}", tag=f"qT{i}") for i in range(2)]
            kT = [singles.tile([128, SK], bf16, name=f"kT{i}", tag=f"kT{i}") for i in range(2)]
            v_sb = singles.tile([128, KT, E], bf16, tag="v_sb")

            partials_in = dram.tile([NPOST, 2, 257, F], bf16)
            partials_out = dram.tile([NPOST, 257, F], bf16)

            with tc.tile_pool(name="nat", bufs=4) as nat, \
                 tc.tile_pool(name="trsb", bufs=6) as trsb, \
                 tc.tile_pool(name="ps_tr", bufs=3, space="PSUM") as ps_tr, \
                 tc.tile_pool(name="ps_mm", bufs=3, space="PSUM") as ps_mm:

                def load_transposed(src_ap, dst_tiles):
                    x_nat = nat.tile([128, 4, E], bf16, tag="nat", name="x_nat")
                    nc.gpsimd.dma_start(out=x_nat[:],
                                        in_=src_ap.rearrange("(t p) e -> p t e", p=128))
                    for et in range(2):
                        pst = ps_tr.tile([128, F], bf16, tag="ps_tr", name="pst")
                        for t in range(4):
                            nc.tensor.transpose(
                                pst[:, 128 * t:128 * (t + 1)],
                                x_nat[:, t, 128 * et:128 * (et + 1)], ident_bf[:])
                        nc.vector.tensor_copy(out=dst_tiles[et][:], in_=pst[:])

                for sc in range(4):
                    zT = [trsb.tile([128, F], bf16, tag="xT", name=f"zT{et}")
                          for et in range(2)]
                    load_transposed(z_ext[512 * sc:512 * (sc + 1), :], zT)
                    for ft in range(2):
                        psk = ps_mm.tile([128, F], f32, tag="ps_mm", name="psk")
                        for et in range(2):
                            nc.tensor.matmul(psk[:], wT["k", et][:, 128 * ft:128 * (ft + 1)],
                                             zT[et][:], start=(et == 0), stop=(et == 1))
                        nc.vector.tensor_copy(out=kT[ft][:, 512 * sc:512 * (sc + 1)],
                                              in_=psk[:])
                    for t in range(4):
                        psv = ps_mm.tile([128, E], f32, tag="ps_mm", name="psv",
                                         padded_shape=[128, 512])
                        for et in range(2):
                            nc.tensor.matmul(psv[:], zT[et][:, 128 * t:128 * (t + 1)],
                                             wT["v", et][:], start=(et == 0), stop=(et == 1))
                        nc.vector.tensor_copy(out=v_sb[:, 4 * sc + t, :], in_=psv[:])

                for j in (3, 7, 2, 6, 1, 5, 0, 4):
                    xT = [trsb.tile([128, F], bf16, tag="xT", name=f"xT{et}")
                          for et in range(2)]
                    load_transposed(x_ext[512 * j:512 * (j + 1), :], xT)
                    for ft in range(2):
                        psq = ps_mm.tile([128, F], f32, tag="ps_mm", name="psq")
                        for et in range(2):
                            nc.tensor.matmul(psq[:], wT["q", et][:, 128 * ft:128 * (ft + 1)],
                                             xT[et][:], start=(et == 0), stop=(et == 1))
                        nc.scalar.activation(out=qT[ft][:, 512 * j:512 * (j + 1)],
                                             in_=psq[:], func=Ident,
                                             bias=bqs[:, ft:ft + 1],
                                             scale=1.0 / float(np.sqrt(E)))

            with tc.tile_pool(name="pT", bufs=6) as pTp, \
                 tc.tile_pool(name="partsb", bufs=4) as partsb, \
                 tc.tile_pool(name="post", bufs=2) as post, \
                 tc.tile_pool(name="ps_s", bufs=3, space="PSUM") as ps_s, \
                 tc.tile_pool(name="ps_o", bufs=2, space="PSUM") as ps_o, \
                 tc.tile_pool(name="ps_rs", bufs=1, space="PSUM") as ps_rs:

                def attn_chunk(j, pair, half):
                    nkt = 2 * (j + 1)
                    pso = ps_o.tile([128, 2 * F], f32, tag="ps_o", name="pso")
                    psr = ps_rs.tile([128, F], f32, tag="ps_rs", name="psr")
                    for ll in range(nkt):
                        pss = ps_s.tile([128, F], f32, tag="ps_s", name="pss")
                        for et in range(2):
                            nc.tensor.matmul(pss[:], kT[et][:, 128 * ll:128 * (ll + 1)],
                                             qT[et][:, 512 * j:512 * (j + 1)],
                                             start=(et == 0), stop=(et == 1))
                        pT = pTp.tile([128, F], bf16, tag="pT", name="pT")
                        nc.scalar.activation(out=pT[:], in_=pss[:], func=Exp)
                        if ll >= nkt - 2:
                            nc.vector.tensor_mul(pT[:], pT[:],
                                                 maskt[:, ll - (nkt - 2), :])
                        for ft in range(2):
                            nc.tensor.matmul(pso[:, F * ft:F * (ft + 1)],
                                             v_sb[:, ll, 128 * ft:128 * (ft + 1)],
                                             pT[:], start=(ll == 0), stop=(ll == nkt - 1),
                                             skip_group_check=True)
                        nc.tensor.matmul(psr[:], ones_r[:], pT[:],
                                         start=(ll == 0), stop=(ll == nkt - 1),
                                         skip_group_check=True)
                    po_sb = partsb.tile([128, 2 * F], bf16, tag="po_sb", name="po_sb")
                    nc.scalar.activation(out=po_sb[:], in_=pso[:], func=Copy)
                    pr_sb = partsb.tile([1, F], bf16, tag="pr_sb", name="pr_sb")
                    nc.vector.tensor_copy(out=pr_sb[:], in_=psr[0:1, :])
                    for ft in range(2):
                        nc.sync.dma_start(
                            out=partials_in[pair, half, 128 * ft:128 * (ft + 1), :],
                            in_=po_sb[:, F * ft:F * (ft + 1)])
                    nc.sync.dma_start(out=partials_in[pair, half, 256, :], in_=pr_sb[0:1, :])

                def post_chunk(c):
                    oT_sb = post.tile([128, 2 * F], bf16, tag="oT_sb", name="oT_sb")
                    for ft in range(2):
                        nc.sync.dma_start(out=oT_sb[:, F * ft:F * (ft + 1)],
                                          in_=partials_out[c, 128 * ft:128 * (ft + 1), :])
                    rs_ld = post.tile([128, 4], bf16, tag="rs_ld", name="rs_ld")
                    nc.sync.dma_start(out=rs_ld[:],
                                      in_=partials_out[c, 256, :].rearrange("(t p) -> p t", p=128))
                    rs_t = post.tile([128, 4], f32, tag="rs_t", name="rs_t")
                    nc.vector.reciprocal(out=rs_t[:], in_=rs_ld[:])
                    onat = post.tile([128, 4, E], f32, tag="onat", name="onat")
                    for t in range(4):
                        pst = ps_s.tile([128, E], bf16, tag="ps_s", name="pstp",
                                        padded_shape=[128, 1024])
                        for ft in range(2):
                            nc.tensor.transpose(
                                pst[:, 128 * ft:128 * (ft + 1)],
                                oT_sb[:, F * ft + 128 * t:F * ft + 128 * (t + 1)],
                                ident_bf[:])
                        nc.scalar.activation(out=onat[:, t, :], in_=pst[:],
                                             func=Copy, scale=rs_t[:, t:t + 1])
                        nc.vector.tensor_add(onat[:, t, :], onat[:, t, :], bv_bc[:])
                    nc.sync.dma_start(
                        out=out_ext[512 * c:512 * (c + 1), :].rearrange(
                            "(t p) e -> p t e", p=128),
                        in_=onat[:])

                prev = None
                for pair in (3, 2, 1, 0):
                    attn_chunk(pair, pair, 0)
                    attn_chunk(NPOST + pair, pair, 1)
                    nc.gpsimd.collective_compute(
                        "ReduceScatter", mybir.AluOpType.add,
                        replica_groups=[[0, 1], [2, 3], [4, 5], [6, 7]],
                        ins=[partials_in[pair].opt()],
                        outs=[partials_out[pair].opt()])
                    if prev is not None:
                        post_chunk(prev)
                    prev = pair
                post_chunk(prev)

    nc.compile()
    return nc


def _get_nc():
    if "nc" not in _COMPILED:
        _COMPILED["nc"] = _build()
    return _COMPILED["nc"]


def kernel(X, Z, mask, Wq, bq, Wk, bk, Wv, bv):
    X = np.asarray(X, dtype=np.float32)
    Z = np.asarray(Z, dtype=np.float32)
    mask_np = np.asarray(mask)

    causal = bool(np.array_equal(
        mask_np != 0, np.tril(np.ones((S, S), dtype=bool))))
    if not causal:
        return _numpy_ref(X, Z, mask_np, Wq, bq, Wk, bk, Wv, bv)

    from concourse.bass_utils import run_bass_kernel_spmd

    nc = _get_nc()

    Wq = np.ascontiguousarray(Wq, dtype=np.float32)
    Wk = np.ascontiguousarray(Wk, dtype=np.float32)
    Wv = np.ascontiguousarray(Wv, dtype=np.float32)
    bqs = (np.asarray(bq, dtype=np.float32) / np.float32(np.sqrt(E))).copy()
    bv = np.ascontiguousarray(bv, dtype=np.float32)
    ones = np.ones((128, 128), dtype=np.float32)

    # masks per parity: last-2 local k-tiles of each chunk; keep iff y >= x+d
    y = np.arange(F)[None, :]
    x = np.arange(128)[:, None]
    masks_par = []
    for p in range(2):
        m = np.stack([(y >= x + 128 * p).astype(np.float32),
                      (y >= x + 256 + 128 * p).astype(np.float32)])
        masks_par.append(np.ascontiguousarray(m))

    in_maps = []
    for c in range(8):
        b, p = c // 2, c % 2
        zb = Z[b].reshape(S // 128, 128, E)
        z_shard = np.ascontiguousarray(zb[p::2].reshape(SK, E))
        in_maps.append({
            "x": np.ascontiguousarray(X[b]),
            "z": z_shard,
            "wq": Wq, "wk": Wk, "wv": Wv,
            "bqs": bqs, "bv": bv,
            "masks": masks_par[p],
            "ones": ones,
        })

    res = run_bass_kernel_spmd(nc, in_maps, core_ids=list(range(8)))

    out = np.empty((B, S, E), dtype=np.float32)
    for b in range(B):
        out[b, :S // 2] = res.results[2 * b]["out"]
        out[b, S // 2:] = res.results[2 * b + 1]["out"]
    return out


def _numpy_ref(X, Z, mask, Wq, bq, Wk, bk, Wv, bv):
    q = np.einsum("bse,fe->bsf", X, Wq) + bq
    k = np.einsum("bse,fe->bsf", Z, Wk) + bk
    v = np.einsum("bse,fe->bsf", Z, Wv) + bv
    s = np.einsum("bqe,bke->bqk", q, k) / np.sqrt(np.float32(X.shape[-1]))
    s = np.where(mask == 0, -np.inf, s)
    s = s - s.max(axis=-1, keepdims=True)
    p = np.exp(s)
    p /= p.sum(axis=-1, keepdims=True)
    return np.einsum("bqk,bke->bqe", p, v).astype(np.float32)



# revision 3
# speedup vs baseline: 1.2252x; 1.2252x over previous
"""Trainium2 Bass kernel: batched causal attention (B=4, S=4096, E=256, f32).

Sharding: 2 cores per batch element (4 pairs).  Within a pair, K/V rows are
split even/odd at 128-row tile granularity; both cores process all 4096 query
rows against their 2048 K/V rows.  The instruction stream is identical across
cores (pure SPMD, no collectives): each core ships its *unnormalized* partial
O^T (bf16) plus per-chunk exp-row-sum accumulators (f32) to DRAM, and the
host merges the pair (add + normalize + transpose + bias) during unshard.

Device-side design (vs the previous on-device ReduceScatter version):
  - Host pre-transposes and pre-casts X^T, Z^T, W^T to bf16 -> no PE
    transposes and half the input DMA bytes.
  - Scores S^T[k,q] = K^T(stationary) . Q^T with exp on the Scalar engine;
    chunk pairs (2m, 2m+1) run k-tiles in lockstep so one [128,1024]
    activation covers two score tiles.
  - Row-sums accumulate on DVE (even chunks) / GpSimd (odd chunks) instead
    of ones-stationary matmuls: saves ~37k PE cycles/core.
  - No collectives, no post phase, no DRAM partial round-trip: the tail is
    just the last output DMA.  Input DMAs are issued up-front across three
    queues (sync/scalar/gpsimd) and the projections interleave with
    attention so the tensor engine stays busy (p-state stays high).
"""

import numpy as np

B = 4
S = 4096
E = 256
SK = S // 2          # K/V rows per core
KT = SK // 128       # 16 local k-tiles
NCH = S // 512       # 8 q-chunks of 512
F = 512

_COMPILED = {}


def _build():
    import concourse.bass as bass
    import concourse.tile as tile
    from concourse import mybir, bacc

    f32 = mybir.dt.float32
    bf16 = mybir.dt.bfloat16
    Exp = mybir.ActivationFunctionType.Exp
    Copy = mybir.ActivationFunctionType.Copy
    Ident = mybir.ActivationFunctionType.Identity

    nc = bacc.Bacc("TRN2", target_bir_lowering=False, debug=False,
                   enable_asserts=True, num_devices=8)

    xt_ext = nc.dram_tensor("xt", [2, 128, S], bf16, kind="ExternalInput")
    zt_ext = nc.dram_tensor("zt", [2, 128, SK], bf16, kind="ExternalInput")
    wq_ext = nc.dram_tensor("wq", [2, 128, E], bf16, kind="ExternalInput")
    wk_ext = nc.dram_tensor("wk", [2, 128, E], bf16, kind="ExternalInput")
    wv_ext = nc.dram_tensor("wv", [2, 128, E], bf16, kind="ExternalInput")
    bqs_ext = nc.dram_tensor("bqs", [128, 2], f32, kind="ExternalInput")
    masks_ext = nc.dram_tensor("masks", [2, 128, F], bf16, kind="ExternalInput")
    out_ext = nc.dram_tensor("out", [NCH, 2, 128, F], bf16, kind="ExternalOutput")
    acc_ext = nc.dram_tensor("acc", [NCH, 128, F], f32, kind="ExternalOutput")

    with tile.TileContext(nc) as tc:
        with tc.tile_pool(name="singles", bufs=1) as singles, \
             tc.tile_pool(name="pT", bufs=3) as pTp, \
             tc.tile_pool(name="accp", bufs=4) as accp, \
             tc.tile_pool(name="pop", bufs=2) as pop, \
             tc.tile_pool(name="ps_a", bufs=2, space="PSUM") as ps_a, \
             tc.tile_pool(name="ps_o", bufs=2, space="PSUM") as ps_o:

            # ---- persistent SBUF tensors ----------------------------------
            xT = singles.tile([128, 2, S], bf16, tag="xT")
            zT = singles.tile([128, 2, SK], bf16, tag="zT")
            qT = singles.tile([128, 2, S], bf16, tag="qT")
            kT = singles.tile([128, 2, SK], bf16, tag="kT")
            v_sb = singles.tile([128, KT, E], bf16, tag="v_sb")
            wq_sb = singles.tile([128, 2, E], bf16, tag="wq_sb")
            wk_sb = singles.tile([128, 2, E], bf16, tag="wk_sb")
            wv_sb = singles.tile([128, 2, E], bf16, tag="wv_sb")
            bqs = singles.tile([128, 2], f32, tag="bqs")
            maskt = singles.tile([128, 2, F], bf16, tag="maskt")

            # ---- all input DMAs issued up front on three queues -----------
            # scalar queue: small singles (weights first: needed earliest)
            nc.scalar.dma_start(out=wk_sb[:],
                                in_=wk_ext.ap().rearrange("e p f -> p e f"))
            nc.scalar.dma_start(out=wv_sb[:],
                                in_=wv_ext.ap().rearrange("e p f -> p e f"))
            nc.scalar.dma_start(out=wq_sb[:],
                                in_=wq_ext.ap().rearrange("e p f -> p e f"))
            nc.scalar.dma_start(out=bqs[:], in_=bqs_ext[:])
            nc.scalar.dma_start(out=maskt[:],
                                in_=masks_ext.ap().rearrange("m p f -> p m f"))
            # gpsimd queue: Z^T in 4 slices (sc-pair x et)
            for ss in range(2):
                for et in range(2):
                    nc.gpsimd.dma_start(
                        out=zT[:, et, 1024 * ss:1024 * (ss + 1)],
                        in_=zt_ext[et, :, 1024 * ss:1024 * (ss + 1)])
            # sync queue: X^T in 8 slices (chunk-pair x et), ascending order
            for jj in range(4):
                for et in range(2):
                    nc.sync.dma_start(
                        out=xT[:, et, 1024 * jj:1024 * (jj + 1)],
                        in_=xt_ext[et, :, 1024 * jj:1024 * (jj + 1)])

            # ---- interleaved projection + attention -----------------------
            for m in range(4):
                sc = m
                # K projection for z columns [512*sc, 512*(sc+1))
                psk = ps_a.tile([128, 2, F], f32, tag="ps_a", name="psk")
                for g in range(2):
                    for et in range(2):
                        nc.tensor.matmul(psk[:, g, :],
                                         wk_sb[:, et, 128 * g:128 * (g + 1)],
                                         zT[:, et, 512 * sc:512 * (sc + 1)],
                                         start=(et == 0), stop=(et == 1),
                                         skip_group_check=(g == 1))
                nc.vector.tensor_copy(out=kT[:, :, 512 * sc:512 * (sc + 1)],
                                      in_=psk[:])
                # V projection (natural layout) for the same z columns
                psv = ps_a.tile([128, 4, E], f32, tag="ps_a", name="psv")
                for t in range(4):
                    base = 512 * sc + 128 * t
                    for et in range(2):
                        nc.tensor.matmul(psv[:, t, :],
                                         zT[:, et, base:base + 128],
                                         wv_sb[:, et, :],
                                         start=(et == 0), stop=(et == 1),
                                         skip_group_check=(t > 0))
                nc.vector.tensor_copy(out=v_sb[:, 4 * sc:4 * sc + 4, :],
                                      in_=psv[:])
                # Q projection for chunks 2m, 2m+1
                for j in (2 * m, 2 * m + 1):
                    psq = ps_a.tile([128, 2, F], f32, tag="ps_a", name="psq")
                    for g in range(2):
                        for et in range(2):
                            nc.tensor.matmul(psq[:, g, :],
                                             wq_sb[:, et, 128 * g:128 * (g + 1)],
                                             xT[:, et, 512 * j:512 * (j + 1)],
                                             start=(et == 0), stop=(et == 1),
                                             skip_group_check=(g == 1))
                    for g in range(2):
                        nc.scalar.activation(out=qT[:, g, 512 * j:512 * (j + 1)],
                                             in_=psq[:, g, :], func=Ident,
                                             bias=bqs[:, g:g + 1])

                # ---- attention for chunk pair (2m, 2m+1), k-tile lockstep -
                c0, c1 = 2 * m, 2 * m + 1
                n0, n1 = 2 * c0 + 2, 2 * c1 + 2
                pso0 = ps_o.tile([128, 2, F], f32, tag="ps_o", name="pso0")
                pso1 = ps_o.tile([128, 2, F], f32, tag="ps_o", name="pso1")
                acc0 = accp.tile([128, F], f32, tag="acc", name="acc0")
                acc1 = accp.tile([128, F], f32, tag="acc", name="acc1")
                for ll in range(n1):
                    both = ll < n0
                    pss = ps_a.tile([128, 2, F], f32, tag="ps_a", name="pss")
                    if both:
                        for et in range(2):
                            nc.tensor.matmul(pss[:, 0, :],
                                             kT[:, et, 128 * ll:128 * (ll + 1)],
                                             qT[:, et, 512 * c0:512 * (c0 + 1)],
                                             start=(et == 0), stop=(et == 1))
                    for et in range(2):
                        nc.tensor.matmul(pss[:, 1, :],
                                         kT[:, et, 128 * ll:128 * (ll + 1)],
                                         qT[:, et, 512 * c1:512 * (c1 + 1)],
                                         start=(et == 0), stop=(et == 1),
                                         skip_group_check=True)
                    pt = pTp.tile([128, 2, F], bf16, tag="pT", name="pt")
                    if both:
                        nc.scalar.activation(out=pt[:], in_=pss[:], func=Exp)
                    else:
                        nc.scalar.activation(out=pt[:, 1, :], in_=pss[:, 1, :],
                                             func=Exp)
                    # causal masks on the last two live k-tiles of each chunk
                    if both and ll >= 2 * c0:
                        nc.vector.tensor_mul(pt[:, 0, :], pt[:, 0, :],
                                             maskt[:, ll - 2 * c0, :])
                    if ll >= 2 * c1:
                        nc.vector.tensor_mul(pt[:, 1, :], pt[:, 1, :],
                                             maskt[:, ll - 2 * c1, :])
                    # exp row-sum accumulators (DVE for c0, GpSimd for c1)
                    if both:
                        if ll == 0:
                            nc.vector.tensor_copy(out=acc0[:], in_=pt[:, 0, :])
                        else:
                            nc.vector.tensor_add(acc0[:], acc0[:], pt[:, 0, :])
                    if ll == 0:
                        nc.gpsimd.tensor_copy(out=acc1[:], in_=pt[:, 1, :])
                    else:
                        nc.gpsimd.tensor_add(acc1[:], acc1[:], pt[:, 1, :])
                    # P^T @ V accumulation
                    for ft in range(2):
                        if both:
                            nc.tensor.matmul(pso0[:, ft, :],
                                             v_sb[:, ll, 128 * ft:128 * (ft + 1)],
                                             pt[:, 0, :],
                                             start=(ll == 0), stop=(ll == n0 - 1),
                                             skip_group_check=True)
                        nc.tensor.matmul(pso1[:, ft, :],
                                         v_sb[:, ll, 128 * ft:128 * (ft + 1)],
                                         pt[:, 1, :],
                                         start=(ll == 0), stop=(ll == n1 - 1),
                                         skip_group_check=True)
                    if ll == n0 - 1:
                        po0 = pop.tile([128, 2, F], bf16, tag="po", name="po0")
                        nc.scalar.activation(out=po0[:], in_=pso0[:], func=Copy)
                        for ft in range(2):
                            nc.sync.dma_start(out=out_ext[c0, ft],
                                              in_=po0[:, ft, :])
                        nc.sync.dma_start(out=acc_ext[c0], in_=acc0[:])
                    if ll == n1 - 1:
                        po1 = pop.tile([128, 2, F], bf16, tag="po", name="po1")
                        nc.vector.tensor_copy(out=po1[:], in_=pso1[:])
                        for ft in range(2):
                            nc.sync.dma_start(out=out_ext[c1, ft],
                                              in_=po1[:, ft, :])
                        nc.sync.dma_start(out=acc_ext[c1], in_=acc1[:])

    nc.compile()
    return nc


def _get_nc():
    if "nc" not in _COMPILED:
        _COMPILED["nc"] = _build()
    return _COMPILED["nc"]


def kernel(X, Z, mask, Wq, bq, Wk, bk, Wv, bv):
    import ml_dtypes
    bf16 = ml_dtypes.bfloat16

    X = np.asarray(X, dtype=np.float32)
    Z = np.asarray(Z, dtype=np.float32)
    mask_np = np.asarray(mask)
    Wq = np.asarray(Wq, dtype=np.float32)
    Wk = np.asarray(Wk, dtype=np.float32)
    Wv = np.asarray(Wv, dtype=np.float32)
    bq = np.asarray(bq, dtype=np.float32)
    bk = np.asarray(bk, dtype=np.float32)
    bv = np.asarray(bv, dtype=np.float32)

    causal = bool(np.array_equal(
        mask_np != 0, np.tril(np.ones((S, S), dtype=bool))))
    if not causal:
        return _numpy_ref(X, Z, mask_np, Wq, bq, Wk, bk, Wv, bv)

    from concourse.bass_utils import run_bass_kernel_spmd

    nc = _get_nc()

    rsqE = np.float32(1.0 / np.sqrt(E))
    # W^T in [e, f] layout, bf16; the 1/sqrt(E) score scale folds into Wq/bq
    wq_t = np.ascontiguousarray((Wq.T * rsqE).astype(bf16)).reshape(2, 128, E)
    wk_t = np.ascontiguousarray(Wk.T.astype(bf16)).reshape(2, 128, E)
    wv_t = np.ascontiguousarray(Wv.T.astype(bf16)).reshape(2, 128, E)
    bqs = np.ascontiguousarray((bq * rsqE).reshape(2, 128).T.astype(np.float32))
    # bk is dropped: it adds a per-q constant to every score row -> softmax
    # shift-invariant.  bv is added on the host after normalization.

    # per-parity diagonal masks for the last two live k-tiles of each chunk
    y = np.arange(F)[None, :]
    x = np.arange(128)[:, None]
    masks_par = []
    for p in range(2):
        mstk = np.stack([(y >= x + 128 * p), (y >= x + 256 + 128 * p)])
        masks_par.append(np.ascontiguousarray(mstk.astype(bf16)))

    in_maps = []
    for c in range(8):
        b, p = c // 2, c % 2
        xt = np.ascontiguousarray(X[b].T.astype(bf16)).reshape(2, 128, S)
        zb = Z[b].reshape(S // 128, 128, E)[p::2].reshape(SK, E)
        zt = np.ascontiguousarray(zb.T.astype(bf16)).reshape(2, 128, SK)
        in_maps.append({
            "xt": xt, "zt": zt,
            "wq": wq_t, "wk": wk_t, "wv": wv_t,
            "bqs": bqs, "masks": masks_par[p],
        })

    res = run_bass_kernel_spmd(nc, in_maps, core_ids=list(range(8)))

    out = np.empty((B, S, E), dtype=np.float32)
    for b in range(B):
        r0, r1 = res.results[2 * b], res.results[2 * b + 1]
        num = r0["out"].astype(np.float32) + r1["out"].astype(np.float32)
        den = r0["acc"].sum(axis=1) + r1["acc"].sum(axis=1)   # [NCH, F]
        ob = num.transpose(0, 3, 1, 2).reshape(S, E)          # [q, (ft, pe)]
        out[b] = ob / den.reshape(S, 1) + bv
    return out


def _numpy_ref(X, Z, mask, Wq, bq, Wk, bk, Wv, bv):
    q = np.einsum("bse,fe->bsf", X, Wq) + bq
    k = np.einsum("bse,fe->bsf", Z, Wk) + bk
    v = np.einsum("bse,fe->bsf", Z, Wv) + bv
    s = np.einsum("bqe,bke->bqk", q, k) / np.sqrt(np.float32(X.shape[-1]))
    s = np.where(mask == 0, -np.inf, s)
    s = s - s.max(axis=-1, keepdims=True)
    p = np.exp(s)
    p /= p.sum(axis=-1, keepdims=True)
    return np.einsum("bqk,bke->bqe", p, v).astype(np.float32)


# revision 10
# speedup vs baseline: 1.6578x; 1.3530x over previous
"""Trainium2 Bass kernel: batched causal attention (B=4, S=4096, E=256, f32).

Sharding: 2 cores per batch element (4 pairs).  Within a pair, K/V rows are
split even/odd at 128-row tile granularity; both cores process all 4096 query
rows against their 2048 K/V rows.  The instruction stream is identical across
cores (pure SPMD, no collectives): each core ships its *unnormalized* partial
O^T (bf16) plus per-chunk exp-row-sum accumulators (bf16) to DRAM, and the
host merges the pair (add + normalize + transpose + bias) during unshard.

Device-side design notes:
  - Host pre-transposes and pre-casts X^T, Z^T, W^T to bf16 -> no PE
    transposes and half the input DMA bytes.
  - Scores S^T[k,q] = K^T(stationary) . Q^T; chunk pairs (2m, 2m+1) run
    k-tiles in lockstep so one scalar-engine activation covers both score
    tiles (fewer, fatter activations).
  - Diagonal k-tiles are *narrowed*: only the live column range is computed,
    and the partially-masked leading 128-column block is always the same
    lower-triangular 128x128 pattern (one small DVE multiply).
  - Exp row-sums accumulate on DVE in bf16 (error ~1e-3 of den, well inside
    the 2e-2 budget); the host does the final 128-partition reduction.
  - K/V projections are emitted one pair ahead of use so their PSUM->SBUF
    casts hide under attention, keeping the tensor engine stream dense.
"""

import numpy as np

B = 4
S = 4096
E = 256
SK = S // 2          # K/V rows per core
KT = SK // 128       # 16 local k-tiles
NCH = S // 512       # 8 q-chunks of 512
F = 512

_COMPILED = {}


def _build():
    import concourse.bass as bass
    import concourse.tile as tile
    from concourse import mybir, bacc

    f32 = mybir.dt.float32
    bf16 = mybir.dt.bfloat16
    Exp = mybir.ActivationFunctionType.Exp
    Copy = mybir.ActivationFunctionType.Copy
    Ident = mybir.ActivationFunctionType.Identity

    nc = bacc.Bacc("TRN2", target_bir_lowering=False, debug=False,
                   enable_asserts=True, num_devices=8)

    xt_ext = nc.dram_tensor("xt", [2, 128, S], bf16, kind="ExternalInput")
    zt_ext = nc.dram_tensor("zt", [2, 128, SK], bf16, kind="ExternalInput")
    wq_ext = nc.dram_tensor("wq", [2, 128, E], bf16, kind="ExternalInput")
    wk_ext = nc.dram_tensor("wk", [2, 128, E], bf16, kind="ExternalInput")
    wv_ext = nc.dram_tensor("wv", [2, 128, E], bf16, kind="ExternalInput")
    bqs_ext = nc.dram_tensor("bqs", [128, 2], f32, kind="ExternalInput")
    # diag mask: one parity-encoded 256-col pattern (keep iff u >= 128p + k)
    # serves both diagonal k-tiles of every chunk
    masks_ext = nc.dram_tensor("masks", [128, 256], bf16, kind="ExternalInput")
    out_ext = nc.dram_tensor("out", [NCH, 2, 128, F], bf16, kind="ExternalOutput")
    acc_ext = nc.dram_tensor("acc", [NCH, 128, F], bf16, kind="ExternalOutput")

    with tile.TileContext(nc) as tc:
        with tc.tile_pool(name="singles", bufs=1) as singles, \
             tc.tile_pool(name="pT", bufs=4) as pTp, \
             tc.tile_pool(name="accp", bufs=2) as accp, \
             tc.tile_pool(name="pop", bufs=2) as pop, \
             tc.tile_pool(name="ps_a", bufs=2, space="PSUM") as ps_a, \
             tc.tile_pool(name="ps_o", bufs=2, space="PSUM") as ps_o:

            # ---- persistent SBUF tensors ----------------------------------
            xT = singles.tile([128, 2, S], bf16, tag="xT")
            zT = singles.tile([128, 2, SK], bf16, tag="zT")
            qT = singles.tile([128, 2, S], bf16, tag="qT")
            kT = singles.tile([128, 2, SK], bf16, tag="kT")
            v_sb = singles.tile([128, KT, E], bf16, tag="v_sb")
            wq_sb = singles.tile([128, 2, E], bf16, tag="wq_sb")
            wk_sb = singles.tile([128, 2, E], bf16, tag="wk_sb")
            wv_sb = singles.tile([128, 2, E], bf16, tag="wv_sb")
            bqs = singles.tile([128, 2], f32, tag="bqs")
            maskt = singles.tile([128, 256], bf16, tag="maskt")

            # ---- all input DMAs issued up front ---------------------------
            nc.scalar.dma_start(out=wk_sb[:],
                                in_=wk_ext.ap().rearrange("e p f -> p e f"))
            nc.scalar.dma_start(out=wv_sb[:],
                                in_=wv_ext.ap().rearrange("e p f -> p e f"))
            nc.scalar.dma_start(out=wq_sb[:],
                                in_=wq_ext.ap().rearrange("e p f -> p e f"))
            nc.scalar.dma_start(out=bqs[:], in_=bqs_ext[:])
            nc.scalar.dma_start(out=maskt[:], in_=masks_ext[:])
            # sync queue: z first (first PE work), interleaved with x chunks
            zx_order = [("z", 0), ("x", 0), ("z", 1), ("x", 1), ("x", 2), ("x", 3)]
            for kind, h in zx_order:
                for et in range(2):
                    if kind == "z":
                        nc.sync.dma_start(
                            out=zT[:, et, 1024 * h:1024 * (h + 1)],
                            in_=zt_ext[et, :, 1024 * h:1024 * (h + 1)])
                    else:
                        nc.sync.dma_start(
                            out=xT[:, et, 1024 * h:1024 * (h + 1)],
                            in_=xt_ext[et, :, 1024 * h:1024 * (h + 1)])

            def kv_block(sc):
                psk = ps_a.tile([128, 2 * F], f32, tag="ps_a", name="psk")
                for g in range(2):
                    for et in range(2):
                        nc.tensor.matmul(psk[:, F * g:F * (g + 1)],
                                         wk_sb[:, et, 128 * g:128 * (g + 1)],
                                         zT[:, et, 512 * sc:512 * (sc + 1)],
                                         start=(et == 0), stop=(et == 1),
                                         skip_group_check=(g == 1))
                nc.vector.tensor_copy(out=kT[:, :, 512 * sc:512 * (sc + 1)],
                                      in_=psk[:])
                psv = ps_a.tile([128, 4, E], f32, tag="ps_a", name="psv",
                                padded_shape=[128, 4, E])
                for t in range(4):
                    base = 512 * sc + 128 * t
                    for et in range(2):
                        nc.tensor.matmul(psv[:, t, :],
                                         zT[:, et, base:base + 128],
                                         wv_sb[:, et, :],
                                         start=(et == 0), stop=(et == 1),
                                         skip_group_check=(t > 0))
                nc.vector.tensor_copy(out=v_sb[:, 4 * sc:4 * sc + 4, :],
                                      in_=psv[:])

            def q_block(j):
                psq = ps_a.tile([128, 2 * F], f32, tag="ps_a", name="psq")
                for g in range(2):
                    for et in range(2):
                        nc.tensor.matmul(psq[:, F * g:F * (g + 1)],
                                         wq_sb[:, et, 128 * g:128 * (g + 1)],
                                         xT[:, et, 512 * j:512 * (j + 1)],
                                         start=(et == 0), stop=(et == 1),
                                         skip_group_check=(g == 1))
                for g in range(2):
                    nc.scalar.activation(out=qT[:, g, 512 * j:512 * (j + 1)],
                                         in_=psq[:, F * g:F * (g + 1)],
                                         func=Ident, bias=bqs[:, g:g + 1])

            def attn_pair(m):
                c0, c1 = 2 * m, 2 * m + 1
                n0, n1 = 2 * c0 + 2, 2 * c1 + 2
                pso0 = ps_o.tile([128, 2 * F], f32, tag="ps_o", name="pso0")
                pso1 = ps_o.tile([128, 2 * F], f32, tag="ps_o", name="pso1")
                acc = accp.tile([128, 2 * F], bf16, tag="acc", name="acc")

                for ll in range(n1):
                    both = ll < n0
                    pss = ps_a.tile([128, 2 * F], f32, tag="ps_a", name="pss")
                    pt = pTp.tile([128, 2 * F], bf16, tag="pT", name="pt")
                    regions = []   # (chunk, col base, live col start)
                    for ci, base in ([(c0, 0)] if both else []) + [(c1, F)]:
                        # diag tile index m_ = ll - 2*ci: second diag tile is
                        # live only from col 256 on (both parities)
                        cs = 256 if ll == 2 * ci + 1 else 0
                        regions.append((ci, base, cs))
                    lo_all = regions[0][1] + regions[0][2]
                    hi = 2 * F
                    for ci, base, cs in regions:
                        for et in range(2):
                            nc.tensor.matmul(
                                pss[:, base + cs:base + F],
                                kT[:, et, 128 * ll:128 * (ll + 1)],
                                qT[:, et, 512 * ci + cs:512 * (ci + 1)],
                                start=(et == 0), stop=(et == 1),
                                skip_group_check=True)
                    nc.scalar.activation(out=pt[:, lo_all:hi],
                                         in_=pss[:, lo_all:hi], func=Exp)
                    # diag masks: one 256-wide parity-encoded pattern
                    for ci, base, cs in regions:
                        if ll >= 2 * ci:
                            nc.vector.tensor_mul(
                                pt[:, base + cs:base + cs + 256],
                                pt[:, base + cs:base + cs + 256],
                                maskt[:])
                    # exp row-sum accumulation (bf16, DVE)
                    if ll == 0:
                        nc.vector.tensor_copy(out=acc[:], in_=pt[:])
                    else:
                        nc.vector.tensor_add(acc[:, lo_all:hi],
                                             acc[:, lo_all:hi],
                                             pt[:, lo_all:hi])
                    # P^T @ V accumulation
                    for ci, base, cs in regions:
                        pso = pso0 if ci == c0 and both else pso1
                        last = (ll == (n0 - 1 if ci == c0 and both else n1 - 1))
                        for ft in range(2):
                            nc.tensor.matmul(
                                pso[:, F * ft + cs:F * (ft + 1)],
                                v_sb[:, ll, 128 * ft:128 * (ft + 1)],
                                pt[:, base + cs:base + F],
                                start=(ll == 0), stop=last,
                                skip_group_check=True)
                    if ll == n0 - 1:
                        po0 = pop.tile([128, 2 * F], bf16, tag="po", name="po0")
                        nc.scalar.activation(out=po0[:], in_=pso0[:], func=Copy)
                        for ft in range(2):
                            nc.sync.dma_start(out=out_ext[c0, ft],
                                              in_=po0[:, F * ft:F * (ft + 1)])
                        nc.sync.dma_start(out=acc_ext[c0], in_=acc[:, 0:F])
                    if ll == n1 - 1:
                        po1 = pop.tile([128, 2 * F], bf16, tag="po", name="po1")
                        nc.vector.tensor_copy(out=po1[:], in_=pso1[:])
                        for ft in range(2):
                            nc.sync.dma_start(out=out_ext[c1, ft],
                                              in_=po1[:, F * ft:F * (ft + 1)])
                        nc.sync.dma_start(out=acc_ext[c1], in_=acc[:, F:2 * F])

            kv_block(0)
            q_block(0)
            q_block(1)
            kv_block(1)
            attn_pair(0)
            kv_block(2)
            q_block(2)
            q_block(3)
            attn_pair(1)
            kv_block(3)
            q_block(4)
            q_block(5)
            attn_pair(2)
            q_block(6)
            q_block(7)
            attn_pair(3)

    nc.compile()
    return nc


def _get_nc():
    if "nc" not in _COMPILED:
        _COMPILED["nc"] = _build()
    return _COMPILED["nc"]


def kernel(X, Z, mask, Wq, bq, Wk, bk, Wv, bv):
    import ml_dtypes
    bf16 = ml_dtypes.bfloat16

    X = np.asarray(X, dtype=np.float32)
    Z = np.asarray(Z, dtype=np.float32)
    mask_np = np.asarray(mask)
    Wq = np.asarray(Wq, dtype=np.float32)
    Wk = np.asarray(Wk, dtype=np.float32)
    Wv = np.asarray(Wv, dtype=np.float32)
    bq = np.asarray(bq, dtype=np.float32)
    bv = np.asarray(bv, dtype=np.float32)

    causal = bool(np.array_equal(
        mask_np != 0, np.tril(np.ones((S, S), dtype=bool))))
    if not causal:
        return _numpy_ref(X, Z, mask_np, Wq, bq, Wk, np.asarray(bk), Wv, bv)

    from concourse.bass_utils import run_bass_kernel_spmd

    nc = _get_nc()

    rsqE = np.float32(1.0 / np.sqrt(E))
    wq_t = np.ascontiguousarray((Wq.T * rsqE).astype(bf16)).reshape(2, 128, E)
    wk_t = np.ascontiguousarray(Wk.T.astype(bf16)).reshape(2, 128, E)
    wv_t = np.ascontiguousarray(Wv.T.astype(bf16)).reshape(2, 128, E)
    bqs = np.ascontiguousarray((bq * rsqE).reshape(2, 128).T.astype(np.float32))
    # bk is dropped: per-q-row constant in the scores -> softmax invariant.

    u = np.arange(256)[None, :]
    x = np.arange(128)[:, None]
    masks_par = [np.ascontiguousarray((u >= 128 * p + x).astype(bf16))
                 for p in range(2)]

    in_maps = []
    for c in range(8):
        b, p = c // 2, c % 2
        xt = np.ascontiguousarray(X[b].T.astype(bf16)).reshape(2, 128, S)
        zb = Z[b].reshape(S // 128, 128, E)[p::2].reshape(SK, E)
        zt = np.ascontiguousarray(zb.T.astype(bf16)).reshape(2, 128, SK)
        in_maps.append({
            "xt": xt, "zt": zt,
            "wq": wq_t, "wk": wk_t, "wv": wv_t,
            "bqs": bqs, "masks": masks_par[p],
        })

    res = run_bass_kernel_spmd(nc, in_maps, core_ids=list(range(8)))

    out = np.empty((B, S, E), dtype=np.float32)
    for b in range(B):
        r0, r1 = res.results[2 * b], res.results[2 * b + 1]
        num = r0["out"].astype(np.float32) + r1["out"].astype(np.float32)
        den = (r0["acc"].astype(np.float32).sum(axis=1)
               + r1["acc"].astype(np.float32).sum(axis=1))  # [NCH, F]
        ob = num.transpose(0, 3, 1, 2).reshape(S, E)
        out[b] = ob / den.reshape(S, 1) + bv
    return out


def _numpy_ref(X, Z, mask, Wq, bq, Wk, bk, Wv, bv):
    q = np.einsum("bse,fe->bsf", X, Wq) + bq
    k = np.einsum("bse,fe->bsf", Z, Wk) + bk
    v = np.einsum("bse,fe->bsf", Z, Wv) + bv
    s = np.einsum("bqe,bke->bqk", q, k) / np.sqrt(np.float32(X.shape[-1]))
    s = np.where(mask == 0, -np.inf, s)
    s = s - s.max(axis=-1, keepdims=True)
    p = np.exp(s)
    p /= p.sum(axis=-1, keepdims=True)
    return np.einsum("bqk,bke->bqe", p, v).astype(np.float32)


# revision 13
# speedup vs baseline: 1.6738x; 1.0097x over previous
"""Trainium2 Bass kernel: batched causal attention (B=4, S=4096, E=256, f32).

Sharding: 2 cores per batch element (4 pairs).  Within a pair, K/V rows are
split even/odd at 128-row tile granularity; both cores process all 4096 query
rows against their 2048 K/V rows.  The instruction stream is identical across
cores (pure SPMD, no collectives): each core ships its *unnormalized* partial
O^T (bf16) plus per-chunk exp-row-sum accumulators (bf16) to DRAM, and the
host merges the pair (add + normalize + transpose + bias) during unshard.

Device-side design notes:
  - Host pre-transposes and pre-casts X^T, Z^T, W^T to bf16 -> no PE
    transposes and half the input DMA bytes.
  - Scores S^T[k,q] = K^T(stationary) . Q^T; chunk pairs (2m, 2m+1) run
    k-tiles in lockstep so one scalar-engine activation covers both score
    tiles (fewer, fatter activations).
  - Diagonal k-tiles are *narrowed*: only the live column range is computed,
    and the partially-masked leading 128-column block is always the same
    lower-triangular 128x128 pattern (one small DVE multiply).
  - Exp row-sums accumulate on DVE in bf16 (error ~1e-3 of den, well inside
    the 2e-2 budget); the host does the final 128-partition reduction.
  - K/V projections are emitted one pair ahead of use so their PSUM->SBUF
    casts hide under attention, keeping the tensor engine stream dense.
"""

import numpy as np

B = 4
S = 4096
E = 256
SK = S // 2          # K/V rows per core
KT = SK // 128       # 16 local k-tiles
NCH = S // 512       # 8 q-chunks of 512
F = 512

_COMPILED = {}


def _build():
    import concourse.bass as bass
    import concourse.tile as tile
    from concourse import mybir, bacc

    f32 = mybir.dt.float32
    bf16 = mybir.dt.bfloat16
    Exp = mybir.ActivationFunctionType.Exp
    Copy = mybir.ActivationFunctionType.Copy
    Ident = mybir.ActivationFunctionType.Identity

    nc = bacc.Bacc("TRN2", target_bir_lowering=False, debug=False,
                   enable_asserts=True, num_devices=8)

    xt_ext = nc.dram_tensor("xt", [2, 128, S], bf16, kind="ExternalInput")
    zt_ext = nc.dram_tensor("zt", [2, 128, SK], bf16, kind="ExternalInput")
    wq_ext = nc.dram_tensor("wq", [2, 128, E], bf16, kind="ExternalInput")
    wk_ext = nc.dram_tensor("wk", [2, 128, E], bf16, kind="ExternalInput")
    wv_ext = nc.dram_tensor("wv", [2, 128, E], bf16, kind="ExternalInput")
    bqs_ext = nc.dram_tensor("bqs", [128, 2], f32, kind="ExternalInput")
    # diag mask: one parity-encoded 256-col pattern (keep iff u >= 128p + k)
    # serves both diagonal k-tiles of every chunk
    masks_ext = nc.dram_tensor("masks", [128, 256], bf16, kind="ExternalInput")
    out_ext = nc.dram_tensor("out", [NCH, 2, 128, F], bf16, kind="ExternalOutput")
    acc_ext = nc.dram_tensor("acc", [NCH, 128, F], bf16, kind="ExternalOutput")

    with tile.TileContext(nc) as tc:
        with tc.tile_pool(name="singles", bufs=1) as singles, \
             tc.tile_pool(name="pT", bufs=6) as pTp, \
             tc.tile_pool(name="accp", bufs=3) as accp, \
             tc.tile_pool(name="pop", bufs=2) as pop, \
             tc.tile_pool(name="ps_a", bufs=2, space="PSUM") as ps_a, \
             tc.tile_pool(name="ps_o", bufs=2, space="PSUM") as ps_o:

            # ---- persistent SBUF tensors ----------------------------------
            xT = singles.tile([128, 2, S], bf16, tag="xT")
            zT = singles.tile([128, 2, SK], bf16, tag="zT")
            qT = singles.tile([128, 2, S], bf16, tag="qT")
            kT = singles.tile([128, 2, SK], bf16, tag="kT")
            v_sb = singles.tile([128, KT, E], bf16, tag="v_sb")
            wq_sb = singles.tile([128, 2, E], bf16, tag="wq_sb")
            wk_sb = singles.tile([128, 2, E], bf16, tag="wk_sb")
            wv_sb = singles.tile([128, 2, E], bf16, tag="wv_sb")
            bqs = singles.tile([128, 2], f32, tag="bqs")
            maskt = singles.tile([128, 256], bf16, tag="maskt")

            # ---- all input DMAs issued up front ---------------------------
            # weights on the scalar HWDGE queue (2 issues, then it's free
            # for activations); the rest on the otherwise-idle gpsimd queue
            nc.scalar.dma_start(out=wk_sb[:],
                                in_=wk_ext.ap().rearrange("e p f -> p e f"))
            nc.scalar.dma_start(out=wv_sb[:],
                                in_=wv_ext.ap().rearrange("e p f -> p e f"))
            nc.gpsimd.dma_start(out=wq_sb[:],
                                in_=wq_ext.ap().rearrange("e p f -> p e f"))
            nc.gpsimd.dma_start(out=bqs[:], in_=bqs_ext[:])
            nc.gpsimd.dma_start(out=maskt[:], in_=masks_ext[:])
            # sync queue: z first (first PE work), interleaved with x chunks
            zx_order = [("z", 0), ("x", 0), ("z", 1), ("x", 1), ("x", 2), ("x", 3)]
            for kind, h in zx_order:
                for et in range(2):
                    if kind == "z":
                        nc.sync.dma_start(
                            out=zT[:, et, 1024 * h:1024 * (h + 1)],
                            in_=zt_ext[et, :, 1024 * h:1024 * (h + 1)])
                    else:
                        nc.sync.dma_start(
                            out=xT[:, et, 1024 * h:1024 * (h + 1)],
                            in_=xt_ext[et, :, 1024 * h:1024 * (h + 1)])

            def kv_block(sc):
                psk = ps_a.tile([128, 2 * F], f32, tag="ps_a", name="psk")
                for g in range(2):
                    for et in range(2):
                        nc.tensor.matmul(psk[:, F * g:F * (g + 1)],
                                         wk_sb[:, et, 128 * g:128 * (g + 1)],
                                         zT[:, et, 512 * sc:512 * (sc + 1)],
                                         start=(et == 0), stop=(et == 1),
                                         skip_group_check=(g == 1))
                nc.vector.tensor_copy(out=kT[:, :, 512 * sc:512 * (sc + 1)],
                                      in_=psk[:])
                psv = ps_a.tile([128, 4, E], f32, tag="ps_a", name="psv",
                                padded_shape=[128, 4, E])
                for t in range(4):
                    base = 512 * sc + 128 * t
                    for et in range(2):
                        nc.tensor.matmul(psv[:, t, :],
                                         zT[:, et, base:base + 128],
                                         wv_sb[:, et, :],
                                         start=(et == 0), stop=(et == 1),
                                         skip_group_check=(t > 0))
                nc.vector.tensor_copy(out=v_sb[:, 4 * sc:4 * sc + 4, :],
                                      in_=psv[:])

            def q_block(j):
                psq = ps_a.tile([128, 2 * F], f32, tag="ps_a", name="psq")
                for g in range(2):
                    for et in range(2):
                        nc.tensor.matmul(psq[:, F * g:F * (g + 1)],
                                         wq_sb[:, et, 128 * g:128 * (g + 1)],
                                         xT[:, et, 512 * j:512 * (j + 1)],
                                         start=(et == 0), stop=(et == 1),
                                         skip_group_check=(g == 1))
                for g in range(2):
                    nc.scalar.activation(out=qT[:, g, 512 * j:512 * (j + 1)],
                                         in_=psq[:, F * g:F * (g + 1)],
                                         func=Ident, bias=bqs[:, g:g + 1])

            def attn_pair(m):
                c0, c1 = 2 * m, 2 * m + 1
                n0, n1 = 2 * c0 + 2, 2 * c1 + 2
                pso0 = ps_o.tile([128, 2 * F], f32, tag="ps_o", name="pso0")
                pso1 = ps_o.tile([128, 2 * F], f32, tag="ps_o", name="pso1")
                acc = accp.tile([128, 2 * F], bf16, tag="acc", name="acc")

                for ll in range(n1):
                    both = ll < n0
                    pss = ps_a.tile([128, 2 * F], f32, tag="ps_a", name="pss")
                    pt = pTp.tile([128, 2 * F], bf16, tag="pT", name="pt")
                    regions = []   # (chunk, col base, live col start)
                    for ci, base in ([(c0, 0)] if both else []) + [(c1, F)]:
                        # diag tile index m_ = ll - 2*ci: second diag tile is
                        # live only from col 256 on (both parities)
                        cs = 256 if ll == 2 * ci + 1 else 0
                        regions.append((ci, base, cs))
                    lo_all = regions[0][1] + regions[0][2]
                    hi = 2 * F
                    for ci, base, cs in regions:
                        for et in range(2):
                            nc.tensor.matmul(
                                pss[:, base + cs:base + F],
                                kT[:, et, 128 * ll:128 * (ll + 1)],
                                qT[:, et, 512 * ci + cs:512 * (ci + 1)],
                                start=(et == 0), stop=(et == 1),
                                skip_group_check=True)
                    nc.scalar.activation(out=pt[:, lo_all:hi],
                                         in_=pss[:, lo_all:hi], func=Exp)
                    # diag masks: one 256-wide parity-encoded pattern
                    for ci, base, cs in regions:
                        if ll >= 2 * ci:
                            nc.vector.tensor_mul(
                                pt[:, base + cs:base + cs + 256],
                                pt[:, base + cs:base + cs + 256],
                                maskt[:])
                    # exp row-sum accumulation (bf16, DVE)
                    if ll == 0:
                        nc.vector.tensor_copy(out=acc[:], in_=pt[:])
                    else:
                        nc.vector.tensor_add(acc[:, lo_all:hi],
                                             acc[:, lo_all:hi],
                                             pt[:, lo_all:hi])
                    # P^T @ V accumulation
                    for ci, base, cs in regions:
                        pso = pso0 if ci == c0 and both else pso1
                        last = (ll == (n0 - 1 if ci == c0 and both else n1 - 1))
                        for ft in range(2):
                            nc.tensor.matmul(
                                pso[:, F * ft + cs:F * (ft + 1)],
                                v_sb[:, ll, 128 * ft:128 * (ft + 1)],
                                pt[:, base + cs:base + F],
                                start=(ll == 0), stop=last,
                                skip_group_check=True)
                    if ll == n0 - 1:
                        po0 = pop.tile([128, 2 * F], bf16, tag="po", name="po0")
                        nc.scalar.activation(out=po0[:], in_=pso0[:], func=Copy)
                        for ft in range(2):
                            nc.sync.dma_start(out=out_ext[c0, ft],
                                              in_=po0[:, F * ft:F * (ft + 1)])
                        nc.sync.dma_start(out=acc_ext[c0], in_=acc[:, 0:F])
                    if ll == n1 - 1:
                        po1 = pop.tile([128, 2 * F], bf16, tag="po", name="po1")
                        nc.vector.tensor_copy(out=po1[:], in_=pso1[:])
                        for ft in range(2):
                            nc.sync.dma_start(out=out_ext[c1, ft],
                                              in_=po1[:, F * ft:F * (ft + 1)])
                        nc.sync.dma_start(out=acc_ext[c1], in_=acc[:, F:2 * F])

            kv_block(0)
            q_block(0)
            q_block(1)
            kv_block(1)
            q_block(2)
            q_block(3)
            attn_pair(0)
            kv_block(2)
            q_block(4)
            q_block(5)
            attn_pair(1)
            kv_block(3)
            q_block(6)
            q_block(7)
            attn_pair(2)
            attn_pair(3)

    nc.compile()
    return nc


def _get_nc():
    if "nc" not in _COMPILED:
        _COMPILED["nc"] = _build()
    return _COMPILED["nc"]


def kernel(X, Z, mask, Wq, bq, Wk, bk, Wv, bv):
    import ml_dtypes
    bf16 = ml_dtypes.bfloat16

    X = np.asarray(X, dtype=np.float32)
    Z = np.asarray(Z, dtype=np.float32)
    mask_np = np.asarray(mask)
    Wq = np.asarray(Wq, dtype=np.float32)
    Wk = np.asarray(Wk, dtype=np.float32)
    Wv = np.asarray(Wv, dtype=np.float32)
    bq = np.asarray(bq, dtype=np.float32)
    bv = np.asarray(bv, dtype=np.float32)

    causal = bool(np.array_equal(
        mask_np != 0, np.tril(np.ones((S, S), dtype=bool))))
    if not causal:
        return _numpy_ref(X, Z, mask_np, Wq, bq, Wk, np.asarray(bk), Wv, bv)

    from concourse.bass_utils import run_bass_kernel_spmd

    nc = _get_nc()

    rsqE = np.float32(1.0 / np.sqrt(E))
    wq_t = np.ascontiguousarray((Wq.T * rsqE).astype(bf16)).reshape(2, 128, E)
    wk_t = np.ascontiguousarray(Wk.T.astype(bf16)).reshape(2, 128, E)
    wv_t = np.ascontiguousarray(Wv.T.astype(bf16)).reshape(2, 128, E)
    bqs = np.ascontiguousarray((bq * rsqE).reshape(2, 128).T.astype(np.float32))
    # bk is dropped: per-q-row constant in the scores -> softmax invariant.

    u = np.arange(256)[None, :]
    x = np.arange(128)[:, None]
    masks_par = [np.ascontiguousarray((u >= 128 * p + x).astype(bf16))
                 for p in range(2)]

    in_maps = []
    for c in range(8):
        b, p = c // 2, c % 2
        xt = np.ascontiguousarray(X[b].T.astype(bf16)).reshape(2, 128, S)
        zb = Z[b].reshape(S // 128, 128, E)[p::2].reshape(SK, E)
        zt = np.ascontiguousarray(zb.T.astype(bf16)).reshape(2, 128, SK)
        in_maps.append({
            "xt": xt, "zt": zt,
            "wq": wq_t, "wk": wk_t, "wv": wv_t,
            "bqs": bqs, "masks": masks_par[p],
        })

    res = run_bass_kernel_spmd(nc, in_maps, core_ids=list(range(8)))

    out = np.empty((B, S, E), dtype=np.float32)
    for b in range(B):
        r0, r1 = res.results[2 * b], res.results[2 * b + 1]
        num = r0["out"].astype(np.float32) + r1["out"].astype(np.float32)
        den = (r0["acc"].astype(np.float32).sum(axis=1)
               + r1["acc"].astype(np.float32).sum(axis=1))  # [NCH, F]
        ob = num.transpose(0, 3, 1, 2).reshape(S, E)
        out[b] = ob / den.reshape(S, 1) + bv
    return out


def _numpy_ref(X, Z, mask, Wq, bq, Wk, bk, Wv, bv):
    q = np.einsum("bse,fe->bsf", X, Wq) + bq
    k = np.einsum("bse,fe->bsf", Z, Wk) + bk
    v = np.einsum("bse,fe->bsf", Z, Wv) + bv
    s = np.einsum("bqe,bke->bqk", q, k) / np.sqrt(np.float32(X.shape[-1]))
    s = np.where(mask == 0, -np.inf, s)
    s = s - s.max(axis=-1, keepdims=True)
    p = np.exp(s)
    p /= p.sum(axis=-1, keepdims=True)
    return np.einsum("bqk,bke->bqe", p, v).astype(np.float32)
